# revision 30
# baseline (speedup 1.0000x reference)
"""Trainium2 Bass kernel for BoundaryLoss (data-parallel over batch).

Math (per batch sample b):
  mask  = boundary mask of target = (maxpool5x5(t) != minpool5x5(t)) with
          cv2-style clipped windows (OOB ignored).  Equals the reference's
          per-class dilate/erode union because a 5x5 window is non-uniform
          iff some class boundary passes through it.
  ce    = logsumexp_c(pred) - pred[t]
  wsum  = sum(mask * ce);  msum = sum(mask)
  per_sample = msum > 0 ? wsum/max(msum,1) : wsum/(H*W);  out = mean_b

Device algorithm (one sample per core), v3:
  - pred streams in "layout B" [128, (4 rows, 512)] (partition p = rows
    4p..4p+3) giving 8KB-contiguous DMA runs.  The pred chunk DMAs are
    the only traffic on the sync HWDGE ring and are issued first; both
    target loads are casting SWDGE DMAs on gpsimd (int32->fp16), so no
    compute engine spends time on casts and the pred stream starts at
    t~=0.
  - S = sum_c exp(pred_c): exp on ACT (fp16 out), summed over classes by
    identity-matmul PSUM accumulation on TensorE (4 banks).
  - picked = pred[t]: per class, eq=(t==c) on DVE 4x; rows 0-2 gathered
    via eq*e mult + identity-matmul PSUM accumulation (3 banks); row 3
    via copy_predicated into SBUF (quarter-FD but 1 op).
  - boundary mask via adjacency edge maps (NO transposes, NO min/max
    pools): a 5x5 window is uniform iff no horizontal-adjacent pair
    differs in rows[r+-2] x cols[c-2..c+1] and no vertical-adjacent pair
    differs in rows[r-2..r+1] x cols[c+-2].  eh/ev neq maps + column box
    sums on DVE (shifted adds); row box sums as banded-weight matmuls
    into one rotating PSUM bank; mask = Sign(B) on ACT straight from
    PSUM.  Row-shift for ev and the layout A->B bounce ride SWDGE.
  - finals per row-group: Ln(S), Ln(G) on ACT; mask-weighted stt accums
    on DVE; partition-reduce via ones-matmuls; DMA out [1,32].
Host combines the per-core outputs.
"""

import numpy as np

B = 8
C = 21
H = 512
W = 512
N_CORES = 8
CHUNK = 2  # pred planes per DMA
EW = 516  # padded width of edge-map tiles; data cols [2, 514)
G4 = 4  # row groups (H = G4 * 128)

_CACHE = {}


def _build_nc():
    from contextlib import ExitStack

    import concourse.bacc as bacc
    import concourse.tile as tile
    from concourse import mybir
    from concourse.masks import make_identity

    dt = mybir.dt
    Alu = mybir.AluOpType
    Act = mybir.ActivationFunctionType

    nc = bacc.Bacc("TRN2", target_bir_lowering=False, debug=False,
                   num_devices=N_CORES)

    pred = nc.dram_tensor("pred", [C, H, W], dt.float32, kind="ExternalInput")
    target = nc.dram_tensor("target", [H, W], dt.int32, kind="ExternalInput")
    out = nc.dram_tensor("out", [1, 32], dt.float32, kind="ExternalOutput")

    with tile.TileContext(nc) as tc, ExitStack() as ctx:
        consts = ctx.enter_context(tc.tile_pool(name="consts", bufs=1))
        keep = ctx.enter_context(tc.tile_pool(name="keep", bufs=1))
        mp = ctx.enter_context(tc.tile_pool(name="maskpool", bufs=1))
        ppool = ctx.enter_context(tc.tile_pool(name="pp", bufs=5))
        epool = ctx.enter_context(tc.tile_pool(name="ep", bufs=4))
        qpool = ctx.enter_context(tc.tile_pool(name="qp", bufs=2))
        jpool = ctx.enter_context(tc.tile_pool(name="jp", bufs=2))
        opool = ctx.enter_context(tc.tile_pool(name="op", bufs=4))
        fin = ctx.enter_context(tc.tile_pool(name="fin", bufs=1))
        mps = ctx.enter_context(tc.tile_pool(name="mpsum", bufs=1,
                                             space="PSUM"))
        sgp = ctx.enter_context(tc.tile_pool(name="sgpsum", bufs=1,
                                             space="PSUM"))

        # ---------------- pred stream: issue ALL chunk DMAs up front ------
        # class 20 first (small fill chunk), class 19 last (small tail
        # chunk); the target load rides the same sync ring right after the
        # first chunk so nothing delays the stream start
        chunk_lists = [[20], [0]] + [[c, c + 1] for c in range(1, 18, 2)]             + [[19]]
        p_tiles = []
        t32b = mp.tile([128, G4, W], dt.int32, tag="t32b")
        for k, cl in enumerate(chunk_lists):
            nct = len(cl)
            p_t = ppool.tile([128, nct, G4, W], dt.float32, tag="p")
            nc.sync.dma_start(
                out=p_t,
                in_=pred.ap()[cl[0]:cl[0] + nct].rearrange(
                    "c (p r) w -> p c r w", p=128))
            p_tiles.append(p_t)
            if k == 0:
                nc.sync.dma_start(
                    out=t32b,
                    in_=target.ap().rearrange("(p r) w -> p r w", p=128))

        # layout-B tensors
        tb = keep.tile([128, G4, W], dt.float16)      # target as fp16
        maskb = keep.tile([128, G4, W], dt.float16)   # mask
        g_sb = keep.tile([128, 1, W], dt.float16)     # r=3 gather | 1.0
        # cast runs on DVE during its idle head
        nc.vector.tensor_copy(out=tb, in_=t32b)

        # ---------------- constants ----------------
        ident = consts.tile([128, 128], dt.float16)
        make_identity(nc, ident)
        ones = consts.tile([128, 1], dt.float32)
        nc.gpsimd.memset(ones, 1.0)
        st_m = consts.tile([128, 2], dt.float32)
        w1acc = consts.tile([128, 1], dt.float32)
        l2acc = consts.tile([128, 2], dt.float32)
        nc.gpsimd.memset(g_sb, 1.0)

        # banded weights: W[p, i] = 1 iff (i - p) in [lo, hi]
        def band(name, lo, hi):
            w = consts.tile([128, 128], dt.float16, tag=name)
            nc.gpsimd.memset(w, 1.0)
            nc.gpsimd.affine_select(
                out=w, in_=w, compare_op=Alu.is_ge, fill=0.0,
                base=-lo, pattern=[[1, 128]], channel_multiplier=-1)
            nc.gpsimd.affine_select(
                out=w, in_=w, compare_op=Alu.is_ge, fill=0.0,
                base=hi, pattern=[[-1, 128]], channel_multiplier=1)
            return w

        # layout-B row bands.  Image row of (p, r) is 4p+r.  For delta_r =
        # r_in - r_out, the partition band v = p_out - p_in ... solved:
        # horizontal-pair window rows [R-2, R+2]: 4v in [dr-2, dr+2]
        # vertical-pair window rows [R-2, R+1]:   4v in [dr-1, dr+2]
        bands = {}

        def get_band(lo, hi):
            if (lo, hi) not in bands:
                bands[(lo, hi)] = band(f"b{lo}_{hi}", lo, hi)
            return bands[(lo, hi)]

        def vrange(lo4, hi4):
            import math
            lo = math.ceil(lo4 / 4)
            hi = math.floor(hi4 / 4)
            return lo, hi

        B5 = {dr: vrange(dr - 2, dr + 2) for dr in range(-3, 4)}
        B4 = {dr: vrange(dr - 1, dr + 2) for dr in range(-3, 4)}

        # ---------------- mask pipeline tiles (layout B) ----------------
        eh = mp.tile([128, G4, EW], dt.float16, tag="eh")
        ev = mp.tile([128, G4, EW], dt.float16, tag="ev")
        s2 = mp.tile([128, G4, EW], dt.float16, tag="s2")
        s4 = mp.tile([128, G4, EW], dt.float16, tag="s4")
        cs4 = mp.tile([128, G4, EW], dt.float16, tag="cs4")
        cs5 = mp.tile([128, G4, W], dt.float16, tag="cs5")
        for t in (eh, ev):
            nc.gpsimd.memset(t, 0.0)

        bps = mps.tile([128, 512], dt.float32, tag="bps")

        def st_eh():
            # horizontal adjacency edge map (1.0 where neighbors differ)
            nc.vector.tensor_tensor(
                out=eh[:, :, 2:1 + W], in0=tb[:, :, 0:W - 1],
                in1=tb[:, :, 1:W], op=Alu.not_equal)

        def st_ev():
            # vertical pairs: rows r=0..2 are in-partition; the r=3 pair
            # row (4p+4 = next partition's row 0) comes from a PE shift
            # matmul through the rotating PSUM bank
            nc.vector.tensor_tensor(
                out=ev[:, 0:3, 2:2 + W], in0=tb[:, 0:3, :],
                in1=tb[:, 1:4, :], op=Alu.not_equal)
            nc.tensor.matmul(bps, get_band(-1, -1), tb[:, 0, :],
                             start=True, stop=False)
            # partition 127 has no successor row (image bottom): feed its
            # own row back so the neq comes out 0 there
            w127 = consts.tile([128, 128], dt.float16, tag="w127")
            nc.gpsimd.memset(w127, 1.0)
            nc.gpsimd.affine_select(
                out=w127, in_=w127, compare_op=Alu.is_ge, fill=0.0,
                base=0, pattern=[[1, 128]], channel_multiplier=-1)
            nc.gpsimd.affine_select(
                out=w127, in_=w127, compare_op=Alu.is_ge, fill=0.0,
                base=-0, pattern=[[-1, 128]], channel_multiplier=1)
            nc.gpsimd.affine_select(
                out=w127, in_=w127, compare_op=Alu.is_ge, fill=0.0,
                base=-127, pattern=[[0, 128]], channel_multiplier=1)
            nc.tensor.matmul(bps, w127, tb[:, 3, :],
                             start=False, stop=True)
            nc.vector.tensor_tensor(
                out=ev[:, 3:4, 2:2 + W], in0=tb[:, 3:4, :],
                in1=bps, op=Alu.not_equal)

        def st_cs4():
            # cs4[k] = eh[k..k+3] (pairs j in [c-2, c+1] at col c = idx k)
            nc.vector.tensor_tensor(
                out=s2[:, :, 0:EW - 1], in0=eh[:, :, 0:EW - 1],
                in1=eh[:, :, 1:EW], op=Alu.add)
            nc.vector.tensor_tensor(
                out=cs4[:, :, 0:EW - 3], in0=s2[:, :, 0:EW - 3],
                in1=s2[:, :, 2:EW - 1], op=Alu.add)

        def st_cs5():
            # cs5[c] = ev[c..c+4] (cols c-2..c+2)
            nc.vector.tensor_tensor(
                out=s2[:, :, 0:EW - 1], in0=ev[:, :, 0:EW - 1],
                in1=ev[:, :, 1:EW], op=Alu.add)
            nc.vector.tensor_tensor(
                out=s4[:, :, 0:EW - 3], in0=s2[:, :, 0:EW - 3],
                in1=s2[:, :, 2:EW - 1], op=Alu.add)
            nc.vector.tensor_tensor(
                out=cs5, in0=s4[:, :, 0:W], in1=ev[:, :, 4:4 + W],
                op=Alu.add)

        def st_box(r_out):
            # row-direction banded sums into the rotating PSUM bank, then
            # threshold straight off PSUM into maskb (sign(B) in {0,1})
            def f():
                mms = []
                for r_in in range(4):
                    dr = r_in - r_out
                    mms.append((get_band(*B5[dr]), cs4[:, r_in, 0:W]))
                    mms.append((get_band(*B4[dr]), cs5[:, r_in, :]))
                for idx, (wgt, mov) in enumerate(mms):
                    nc.tensor.matmul(bps, wgt, mov, start=(idx == 0),
                                     stop=(idx == len(mms) - 1))
                nc.scalar.sign(out=maskb[:, r_out, :], in_=bps)
            return f

        def st_msum(h):
            # split into halves so each fits the ACT slack of one chunk
            def f():
                junk_m = mp.tile([128, 2, W], dt.float16, tag="junkm")
                nc.scalar.activation(out=junk_m,
                                     in_=maskb[:, 2 * h:2 * h + 2, :],
                                     func=Act.Copy,
                                     accum_out=st_m[:, h:h + 1])
            return f

        sched = {
            1: [st_eh, st_ev],
            2: [st_cs4, st_cs5],
            3: [st_box(0), st_box(1)],
            4: [st_box(2), st_box(3)],
            7: [st_msum(0)],
            8: [st_msum(1)],
        }

        # ---------------- class loop (layout B), stages interleaved -------
        s_ps = sgp.tile([128, G4, W], dt.float32, tag="s")
        g_ps = sgp.tile([128, 3, W], dt.float32, tag="g")

        first_c = chunk_lists[0][0]
        last_c = chunk_lists[-1][-1]
        eq_last = keep.tile([128, G4, W], dt.uint16)
        for k, cl in enumerate(chunk_lists):
            for st in sched.get(k, []):
                st()
            if k == 9:
                # precompute the tail class's eq during a mid-stream gap
                nc.vector.tensor_scalar(
                    out=eq_last, in0=tb, scalar1=float(last_c), scalar2=None,
                    op0=Alu.is_equal)
            nct = len(cl)
            p_t = p_tiles[k]
            e_t = epool.tile([128, nct, G4, W], dt.float16, tag="e")
            if k == len(chunk_lists) - 1:
                nc.scalar.activation(out=e_t[:, :, 0:2, :],
                                     in_=p_t[:, :, 0:2, :], func=Act.Exp)
                nc.scalar.activation(out=e_t[:, :, 2:4, :],
                                     in_=p_t[:, :, 2:4, :], func=Act.Exp)
            else:
                nc.scalar.activation(out=e_t, in_=p_t, func=Act.Exp)
            for i in range(nct):
                c = cl[i]
                if c == last_c:
                    eq_t = eq_last
                else:
                    eq_t = qpool.tile([128, G4, W], dt.uint16, tag="q")
                    nc.vector.tensor_scalar(
                        out=eq_t, in0=tb, scalar1=float(c), scalar2=None,
                        op0=Alu.is_equal)
                # rows 0..2: gather via multiply + identity matmul
                o_t = opool.tile([128, 3, W], dt.float16, tag="o")
                nc.vector.tensor_tensor(
                    out=o_t, in0=eq_t[:, 0:3, :], in1=e_t[:, i, 0:3, :],
                    op=Alu.mult)
                # row 3: gather via predicated overwrite (1x but quarter-FD)
                nc.vector.copy_predicated(out=g_sb[:, 0, :],
                                          mask=eq_t[:, 3, :],
                                          data=e_t[:, i, 3, :])
                for j in range(4):
                    nc.tensor.matmul(
                        s_ps[:, j, :], ident, e_t[:, i, j, :],
                        start=(c == first_c), stop=(c == last_c))
                for j in range(3):
                    nc.tensor.matmul(
                        g_ps[:, j, :], ident, o_t[:, j, :],
                        start=(c == first_c), stop=(c == last_c))

        # ---------------- finals ----------------
        l1 = fin.tile([128, G4, W], dt.float16, tag="l1")
        nc.scalar.activation(out=l1, in_=s_ps, func=Act.Ln)
        j1 = jpool.tile([128, G4, W], dt.float16, tag="junkw")
        nc.vector.scalar_tensor_tensor(
            out=j1, in0=l1, scalar=0.0, in1=maskb,
            op0=Alu.add, op1=Alu.mult, accum_out=w1acc[:, 0:1])
        lg = fin.tile([128, 3, W], dt.float16, tag="lg")
        nc.scalar.activation(out=lg, in_=g_ps, func=Act.Ln)
        j2 = jpool.tile([128, 3, W], dt.float16, tag="junkl")
        nc.vector.scalar_tensor_tensor(
            out=j2, in0=lg, scalar=0.0, in1=maskb[:, 0:3, :],
            op0=Alu.add, op1=Alu.mult, accum_out=l2acc[:, 0:1])
        lg4 = fin.tile([128, 1, W], dt.float16, tag="lg4")
        nc.scalar.activation(out=lg4, in_=g_sb, func=Act.Ln)
        j3 = jpool.tile([128, 1, W], dt.float16, tag="junk4")
        nc.vector.scalar_tensor_tensor(
            out=j3, in0=lg4, scalar=0.0, in1=maskb[:, 3:4, :],
            op0=Alu.add, op1=Alu.mult, accum_out=l2acc[:, 1:2])

        # partition reductions — reuse the S bank (fully consumed by l1)
        red = s_ps[0:1, 0, 0:32]
        nc.tensor.matmul(red[:, 0:1], ones, w1acc[:, 0:1], start=True,
                         stop=True)
        nc.tensor.matmul(red[:, 4:6], ones, l2acc[:, 0:2], start=True,
                         stop=True)
        nc.tensor.matmul(red[:, 8:10], ones, st_m, start=True, stop=True)
        outsb = consts.tile([1, 32], dt.float32)
        nc.vector.memset(outsb, 0.0)
        nc.vector.tensor_copy(out=outsb[:, 0:10], in_=red[:, 0:10])
        nc.sync.dma_start(out=out.ap(), in_=outsb)

    nc.compile()
    return nc


def get_nc():
    if "nc" not in _CACHE:
        _CACHE["nc"] = _build_nc()
    return _CACHE["nc"]


def _combine(outs):
    """outs: list of per-core [1,32] float32 -> scalar loss."""
    per_sample = []
    for o in outs:
        w1 = float(o[0, 0])
        l2 = float(o[0, 4:6].sum())
        msum = float(o[0, 8:10].sum())
        wsum = w1 - l2
        if msum > 0:
            per_sample.append(wsum / max(msum, 1.0))
        else:
            per_sample.append(wsum / float(H * W))
    return np.float32(np.mean(per_sample))


def kernel(pred, target):
    from concourse.bass_utils import run_bass_kernel_spmd

    pred = np.ascontiguousarray(pred, dtype=np.float32)
    target = np.ascontiguousarray(target, dtype=np.int32)
    assert pred.shape == (B, C, H, W) and target.shape == (B, H, W)

    nc = get_nc()
    in_maps = [{"pred": pred[b], "target": target[b]} for b in range(B)]
    res = run_bass_kernel_spmd(nc, in_maps, core_ids=list(range(N_CORES)))
    outs = [res.results[b]["out"] for b in range(B)]
    return np.asarray(_combine(outs), dtype=np.float32)


# revision 31
# speedup vs baseline: 1.0174x; 1.0174x over previous
"""Trainium2 Bass kernel for BoundaryLoss (data-parallel over batch).

Math (per batch sample b):
  mask  = boundary mask of target = (maxpool5x5(t) != minpool5x5(t)) with
          cv2-style clipped windows (OOB ignored).  Equals the reference's
          per-class dilate/erode union because a 5x5 window is non-uniform
          iff some class boundary passes through it.
  ce    = logsumexp_c(pred) - pred[t]
  wsum  = sum(mask * ce);  msum = sum(mask)
  per_sample = msum > 0 ? wsum/max(msum,1) : wsum/(H*W);  out = mean_b

Device algorithm (one sample per core), v3:
  - pred streams in "layout B" [128, (4 rows, 512)] (partition p = rows
    4p..4p+3) giving 8KB-contiguous DMA runs.  The pred chunk DMAs are
    the only traffic on the sync HWDGE ring and are issued first; both
    target loads are casting SWDGE DMAs on gpsimd (int32->fp16), so no
    compute engine spends time on casts and the pred stream starts at
    t~=0.
  - S = sum_c exp(pred_c): exp on ACT (fp16 out), summed over classes by
    identity-matmul PSUM accumulation on TensorE (4 banks).
  - picked = pred[t]: per class, eq=(t==c) on DVE 4x; rows 0-2 gathered
    via eq*e mult + identity-matmul PSUM accumulation (3 banks); row 3
    via copy_predicated into SBUF (quarter-FD but 1 op).
  - boundary mask via adjacency edge maps (NO transposes, NO min/max
    pools): a 5x5 window is uniform iff no horizontal-adjacent pair
    differs in rows[r+-2] x cols[c-2..c+1] and no vertical-adjacent pair
    differs in rows[r-2..r+1] x cols[c+-2].  eh/ev neq maps + column box
    sums on DVE (shifted adds); row box sums as banded-weight matmuls
    into one rotating PSUM bank; mask = Sign(B) on ACT straight from
    PSUM.  Row-shift for ev and the layout A->B bounce ride SWDGE.
  - finals per row-group: Ln(S), Ln(G) on ACT; mask-weighted stt accums
    on DVE; partition-reduce via ones-matmuls; DMA out [1,32].
Host combines the per-core outputs.
"""

import numpy as np

B = 8
C = 21
H = 512
W = 512
N_CORES = 8
CHUNK = 2  # pred planes per DMA
EW = 516  # padded width of edge-map tiles; data cols [2, 514)
G4 = 4  # row groups (H = G4 * 128)

_CACHE = {}


def _build_nc():
    from contextlib import ExitStack

    import concourse.bacc as bacc
    import concourse.tile as tile
    from concourse import mybir
    from concourse.masks import make_identity

    dt = mybir.dt
    Alu = mybir.AluOpType
    Act = mybir.ActivationFunctionType

    nc = bacc.Bacc("TRN2", target_bir_lowering=False, debug=False,
                   num_devices=N_CORES)

    pred = nc.dram_tensor("pred", [C, H, W], dt.float32, kind="ExternalInput")
    target = nc.dram_tensor("target", [H, W], dt.int32, kind="ExternalInput")
    out = nc.dram_tensor("out", [1, 32], dt.float32, kind="ExternalOutput")

    with tile.TileContext(nc) as tc, ExitStack() as ctx:
        consts = ctx.enter_context(tc.tile_pool(name="consts", bufs=1))
        keep = ctx.enter_context(tc.tile_pool(name="keep", bufs=1))
        mp = ctx.enter_context(tc.tile_pool(name="maskpool", bufs=1))
        ppool = ctx.enter_context(tc.tile_pool(name="pp", bufs=4))
        epool = ctx.enter_context(tc.tile_pool(name="ep", bufs=4))
        qpool = ctx.enter_context(tc.tile_pool(name="qp", bufs=2))
        jpool = ctx.enter_context(tc.tile_pool(name="jp", bufs=2))
        opool = ctx.enter_context(tc.tile_pool(name="op", bufs=4))
        fin = ctx.enter_context(tc.tile_pool(name="fin", bufs=1))
        mps = ctx.enter_context(tc.tile_pool(name="mpsum", bufs=1,
                                             space="PSUM"))
        sgp = ctx.enter_context(tc.tile_pool(name="sgpsum", bufs=1,
                                             space="PSUM"))

        # ---------------- pred stream: issue ALL chunk DMAs up front ------
        # class 20 first (small fill chunk), class 19 last (small tail
        # chunk); the target load rides the same sync ring right after the
        # first chunk so nothing delays the stream start
        chunk_lists = [[20], [0]] + [[c, c + 1] for c in range(1, 18, 2)]             + [[19]]
        p_tiles = []
        t32b = mp.tile([128, G4, W], dt.int32, tag="t32b")
        for k, cl in enumerate(chunk_lists):
            nct = len(cl)
            p_t = ppool.tile([128, nct, G4, W], dt.float32, tag="p")
            nc.sync.dma_start(
                out=p_t,
                in_=pred.ap()[cl[0]:cl[0] + nct].rearrange(
                    "c (p r) w -> p c r w", p=128))
            p_tiles.append(p_t)
            if k == 0:
                nc.sync.dma_start(
                    out=t32b,
                    in_=target.ap().rearrange("(p r) w -> p r w", p=128))

        # layout-B tensors
        tb = keep.tile([128, G4, W], dt.float16)      # target as fp16
        maskb = keep.tile([128, G4, W], dt.float16)   # mask
        g_sb = keep.tile([128, 1, W], dt.float16)     # r=3 gather | 1.0
        # cast runs on DVE during its idle head
        nc.vector.tensor_copy(out=tb, in_=t32b)

        # ---------------- constants ----------------
        ident = consts.tile([128, 128], dt.float16)
        make_identity(nc, ident)
        ones = consts.tile([128, 1], dt.float32)
        nc.gpsimd.memset(ones, 1.0)
        st_m = consts.tile([128, 2], dt.float32)
        w1acc = consts.tile([128, 1], dt.float32)
        l2acc = consts.tile([128, 2], dt.float32)
        nc.gpsimd.memset(g_sb, 1.0)

        # banded weights: W[p, i] = 1 iff (i - p) in [lo, hi]
        def band(name, lo, hi):
            w = consts.tile([128, 128], dt.float16, tag=name)
            nc.gpsimd.memset(w, 1.0)
            nc.gpsimd.affine_select(
                out=w, in_=w, compare_op=Alu.is_ge, fill=0.0,
                base=-lo, pattern=[[1, 128]], channel_multiplier=-1)
            nc.gpsimd.affine_select(
                out=w, in_=w, compare_op=Alu.is_ge, fill=0.0,
                base=hi, pattern=[[-1, 128]], channel_multiplier=1)
            return w

        # layout-B row bands.  Image row of (p, r) is 4p+r.  For delta_r =
        # r_in - r_out, the partition band v = p_out - p_in ... solved:
        # horizontal-pair window rows [R-2, R+2]: 4v in [dr-2, dr+2]
        # vertical-pair window rows [R-2, R+1]:   4v in [dr-1, dr+2]
        bands = {}

        def get_band(lo, hi):
            if (lo, hi) not in bands:
                bands[(lo, hi)] = band(f"b{lo}_{hi}", lo, hi)
            return bands[(lo, hi)]

        def vrange(lo4, hi4):
            import math
            lo = math.ceil(lo4 / 4)
            hi = math.floor(hi4 / 4)
            return lo, hi

        B5 = {dr: vrange(dr - 2, dr + 2) for dr in range(-3, 4)}
        B4 = {dr: vrange(dr - 1, dr + 2) for dr in range(-3, 4)}

        # ---------------- mask pipeline tiles (layout B) ----------------
        eh = mp.tile([128, G4, EW], dt.float16, tag="eh")
        ev = mp.tile([128, G4, EW], dt.float16, tag="ev")
        s2 = mp.tile([128, G4, EW], dt.float16, tag="s2")
        s4 = mp.tile([128, G4, EW], dt.float16, tag="s4")
        cs4 = mp.tile([128, G4, EW], dt.float16, tag="cs4")
        cs5 = mp.tile([128, G4, W], dt.float16, tag="cs5")
        for t in (eh, ev):
            nc.gpsimd.memset(t, 0.0)

        bps = mps.tile([128, 512], dt.float32, tag="bps")

        def st_eh():
            # horizontal adjacency edge map (1.0 where neighbors differ)
            nc.vector.tensor_tensor(
                out=eh[:, :, 2:1 + W], in0=tb[:, :, 0:W - 1],
                in1=tb[:, :, 1:W], op=Alu.not_equal)

        def st_ev():
            # vertical pairs: rows r=0..2 are in-partition; the r=3 pair
            # row (4p+4 = next partition's row 0) comes from a PE shift
            # matmul through the rotating PSUM bank
            nc.vector.tensor_tensor(
                out=ev[:, 0:3, 2:2 + W], in0=tb[:, 0:3, :],
                in1=tb[:, 1:4, :], op=Alu.not_equal)
            nc.tensor.matmul(bps, get_band(-1, -1), tb[:, 0, :],
                             start=True, stop=False)
            # partition 127 has no successor row (image bottom): feed its
            # own row back so the neq comes out 0 there
            w127 = consts.tile([128, 128], dt.float16, tag="w127")
            nc.gpsimd.memset(w127, 1.0)
            nc.gpsimd.affine_select(
                out=w127, in_=w127, compare_op=Alu.is_ge, fill=0.0,
                base=0, pattern=[[1, 128]], channel_multiplier=-1)
            nc.gpsimd.affine_select(
                out=w127, in_=w127, compare_op=Alu.is_ge, fill=0.0,
                base=-0, pattern=[[-1, 128]], channel_multiplier=1)
            nc.gpsimd.affine_select(
                out=w127, in_=w127, compare_op=Alu.is_ge, fill=0.0,
                base=-127, pattern=[[0, 128]], channel_multiplier=1)
            nc.tensor.matmul(bps, w127, tb[:, 3, :],
                             start=False, stop=True)
            nc.vector.tensor_tensor(
                out=ev[:, 3:4, 2:2 + W], in0=tb[:, 3:4, :],
                in1=bps, op=Alu.not_equal)

        def st_cs4():
            # cs4[k] = eh[k..k+3] (pairs j in [c-2, c+1] at col c = idx k)
            nc.vector.tensor_tensor(
                out=s2[:, :, 0:EW - 1], in0=eh[:, :, 0:EW - 1],
                in1=eh[:, :, 1:EW], op=Alu.add)
            nc.vector.tensor_tensor(
                out=cs4[:, :, 0:EW - 3], in0=s2[:, :, 0:EW - 3],
                in1=s2[:, :, 2:EW - 1], op=Alu.add)

        def st_cs5():
            # cs5[c] = ev[c..c+4] (cols c-2..c+2)
            nc.vector.tensor_tensor(
                out=s2[:, :, 0:EW - 1], in0=ev[:, :, 0:EW - 1],
                in1=ev[:, :, 1:EW], op=Alu.add)
            nc.vector.tensor_tensor(
                out=s4[:, :, 0:EW - 3], in0=s2[:, :, 0:EW - 3],
                in1=s2[:, :, 2:EW - 1], op=Alu.add)
            nc.vector.tensor_tensor(
                out=cs5, in0=s4[:, :, 0:W], in1=ev[:, :, 4:4 + W],
                op=Alu.add)

        def st_box(r_out):
            # row-direction banded sums into the rotating PSUM bank, then
            # threshold straight off PSUM into maskb (sign(B) in {0,1})
            def f():
                mms = []
                for r_in in range(4):
                    dr = r_in - r_out
                    mms.append((get_band(*B5[dr]), cs4[:, r_in, 0:W]))
                    mms.append((get_band(*B4[dr]), cs5[:, r_in, :]))
                for idx, (wgt, mov) in enumerate(mms):
                    nc.tensor.matmul(bps, wgt, mov, start=(idx == 0),
                                     stop=(idx == len(mms) - 1))
                nc.scalar.sign(out=maskb[:, r_out, :], in_=bps)
            return f

        def st_msum(h):
            # split into halves so each fits the ACT slack of one chunk
            def f():
                junk_m = mp.tile([128, 2, W], dt.float16, tag="junkm")
                nc.scalar.activation(out=junk_m,
                                     in_=maskb[:, 2 * h:2 * h + 2, :],
                                     func=Act.Copy,
                                     accum_out=st_m[:, h:h + 1])
            return f

        sched = {
            1: [st_eh, st_ev],
            2: [st_cs4, st_cs5],
            3: [st_box(0), st_box(1)],
            4: [st_box(2), st_box(3)],
            7: [st_msum(0)],
            8: [st_msum(1)],
        }

        # ---------------- class loop (layout B), stages interleaved -------
        s_ps = sgp.tile([128, G4, W], dt.float32, tag="s")
        g_ps = sgp.tile([128, 3, W], dt.float32, tag="g")

        first_c = chunk_lists[0][0]
        last_c = chunk_lists[-1][-1]
        eq_last = keep.tile([128, G4, W], dt.uint16)
        for k, cl in enumerate(chunk_lists):
            for st in sched.get(k, []):
                st()
            if k == 9:
                # precompute the tail class's eq during a mid-stream gap
                nc.vector.tensor_scalar(
                    out=eq_last, in0=tb, scalar1=float(last_c), scalar2=None,
                    op0=Alu.is_equal)
            nct = len(cl)
            p_t = p_tiles[k]
            e_t = epool.tile([128, nct, G4, W], dt.float16, tag="e")
            if k == len(chunk_lists) - 1:
                nc.scalar.activation(out=e_t[:, :, 0:2, :],
                                     in_=p_t[:, :, 0:2, :], func=Act.Exp)
                nc.scalar.activation(out=e_t[:, :, 2:4, :],
                                     in_=p_t[:, :, 2:4, :], func=Act.Exp)
            else:
                nc.scalar.activation(out=e_t, in_=p_t, func=Act.Exp)
            for i in range(nct):
                c = cl[i]
                if c == last_c:
                    eq_t = eq_last
                else:
                    eq_t = qpool.tile([128, G4, W], dt.uint16, tag="q")
                    nc.vector.tensor_scalar(
                        out=eq_t, in0=tb, scalar1=float(c), scalar2=None,
                        op0=Alu.is_equal)
                # rows 0..2: gather via multiply + identity matmul
                o_t = opool.tile([128, 3, W], dt.float16, tag="o")
                nc.vector.tensor_tensor(
                    out=o_t, in0=eq_t[:, 0:3, :], in1=e_t[:, i, 0:3, :],
                    op=Alu.mult)
                # row 3: gather via predicated overwrite (1x but quarter-FD)
                nc.vector.copy_predicated(out=g_sb[:, 0, :],
                                          mask=eq_t[:, 3, :],
                                          data=e_t[:, i, 3, :])
                for j in range(4):
                    nc.tensor.matmul(
                        s_ps[:, j, :], ident, e_t[:, i, j, :],
                        start=(c == first_c), stop=(c == last_c))
                for j in range(3):
                    nc.tensor.matmul(
                        g_ps[:, j, :], ident, o_t[:, j, :],
                        start=(c == first_c), stop=(c == last_c))

        # ---------------- finals ----------------
        l1 = fin.tile([128, G4, W], dt.float16, tag="l1")
        nc.scalar.activation(out=l1, in_=s_ps, func=Act.Ln)
        j1 = jpool.tile([128, G4, W], dt.float16, tag="junkw")
        nc.vector.scalar_tensor_tensor(
            out=j1, in0=l1, scalar=0.0, in1=maskb,
            op0=Alu.add, op1=Alu.mult, accum_out=w1acc[:, 0:1])
        lg = fin.tile([128, 3, W], dt.float16, tag="lg")
        nc.scalar.activation(out=lg, in_=g_ps, func=Act.Ln)
        j2 = jpool.tile([128, 3, W], dt.float16, tag="junkl")
        nc.vector.scalar_tensor_tensor(
            out=j2, in0=lg, scalar=0.0, in1=maskb[:, 0:3, :],
            op0=Alu.add, op1=Alu.mult, accum_out=l2acc[:, 0:1])
        lg4 = fin.tile([128, 1, W], dt.float16, tag="lg4")
        nc.scalar.activation(out=lg4, in_=g_sb, func=Act.Ln)
        j3 = jpool.tile([128, 1, W], dt.float16, tag="junk4")
        nc.vector.scalar_tensor_tensor(
            out=j3, in0=lg4, scalar=0.0, in1=maskb[:, 3:4, :],
            op0=Alu.add, op1=Alu.mult, accum_out=l2acc[:, 1:2])

        # partition reductions — reuse the S bank (fully consumed by l1)
        red = s_ps[0:1, 0, 0:32]
        nc.tensor.matmul(red[:, 0:1], ones, w1acc[:, 0:1], start=True,
                         stop=True)
        nc.tensor.matmul(red[:, 4:6], ones, l2acc[:, 0:2], start=True,
                         stop=True)
        nc.tensor.matmul(red[:, 8:10], ones, st_m, start=True, stop=True)
        outsb = consts.tile([1, 32], dt.float32)
        nc.vector.memset(outsb, 0.0)
        nc.vector.tensor_copy(out=outsb[:, 0:10], in_=red[:, 0:10])
        nc.sync.dma_start(out=out.ap(), in_=outsb)

    nc.compile()
    return nc


def get_nc():
    if "nc" not in _CACHE:
        _CACHE["nc"] = _build_nc()
    return _CACHE["nc"]


def _combine(outs):
    """outs: list of per-core [1,32] float32 -> scalar loss."""
    per_sample = []
    for o in outs:
        w1 = float(o[0, 0])
        l2 = float(o[0, 4:6].sum())
        msum = float(o[0, 8:10].sum())
        wsum = w1 - l2
        if msum > 0:
            per_sample.append(wsum / max(msum, 1.0))
        else:
            per_sample.append(wsum / float(H * W))
    return np.float32(np.mean(per_sample))


def kernel(pred, target):
    from concourse.bass_utils import run_bass_kernel_spmd

    pred = np.ascontiguousarray(pred, dtype=np.float32)
    target = np.ascontiguousarray(target, dtype=np.int32)
    assert pred.shape == (B, C, H, W) and target.shape == (B, H, W)

    nc = get_nc()
    in_maps = [{"pred": pred[b], "target": target[b]} for b in range(B)]
    res = run_bass_kernel_spmd(nc, in_maps, core_ids=list(range(N_CORES)))
    outs = [res.results[b]["out"] for b in range(B)]
    return np.asarray(_combine(outs), dtype=np.float32)


# revision 32
# speedup vs baseline: 1.0176x; 1.0001x over previous
"""Trainium2 Bass kernel for BoundaryLoss (data-parallel over batch).

Math (per batch sample b):
  mask  = boundary mask of target = (maxpool5x5(t) != minpool5x5(t)) with
          cv2-style clipped windows (OOB ignored).  Equals the reference's
          per-class dilate/erode union because a 5x5 window is non-uniform
          iff some class boundary passes through it.
  ce    = logsumexp_c(pred) - pred[t]
  wsum  = sum(mask * ce);  msum = sum(mask)
  per_sample = msum > 0 ? wsum/max(msum,1) : wsum/(H*W);  out = mean_b

Device algorithm (one sample per core), v3:
  - pred streams in "layout B" [128, (4 rows, 512)] (partition p = rows
    4p..4p+3) giving 8KB-contiguous DMA runs.  The pred chunk DMAs are
    the only traffic on the sync HWDGE ring and are issued first; both
    target loads are casting SWDGE DMAs on gpsimd (int32->fp16), so no
    compute engine spends time on casts and the pred stream starts at
    t~=0.
  - S = sum_c exp(pred_c): exp on ACT (fp16 out), summed over classes by
    identity-matmul PSUM accumulation on TensorE (4 banks).
  - picked = pred[t]: per class, eq=(t==c) on DVE 4x; rows 0-2 gathered
    via eq*e mult + identity-matmul PSUM accumulation (3 banks); row 3
    via copy_predicated into SBUF (quarter-FD but 1 op).
  - boundary mask via adjacency edge maps (NO transposes, NO min/max
    pools): a 5x5 window is uniform iff no horizontal-adjacent pair
    differs in rows[r+-2] x cols[c-2..c+1] and no vertical-adjacent pair
    differs in rows[r-2..r+1] x cols[c+-2].  eh/ev neq maps + column box
    sums on DVE (shifted adds); row box sums as banded-weight matmuls
    into one rotating PSUM bank; mask = Sign(B) on ACT straight from
    PSUM.  Row-shift for ev and the layout A->B bounce ride SWDGE.
  - finals per row-group: Ln(S), Ln(G) on ACT; mask-weighted stt accums
    on DVE; partition-reduce via ones-matmuls; DMA out [1,32].
Host combines the per-core outputs.
"""

import numpy as np

B = 8
C = 21
H = 512
W = 512
N_CORES = 8
CHUNK = 2  # pred planes per DMA
EW = 516  # padded width of edge-map tiles; data cols [2, 514)
G4 = 4  # row groups (H = G4 * 128)

_CACHE = {}


def _build_nc():
    from contextlib import ExitStack

    import concourse.bacc as bacc
    import concourse.tile as tile
    from concourse import mybir
    from concourse.masks import make_identity

    dt = mybir.dt
    Alu = mybir.AluOpType
    Act = mybir.ActivationFunctionType

    nc = bacc.Bacc("TRN2", target_bir_lowering=False, debug=False,
                   num_devices=N_CORES)

    pred = nc.dram_tensor("pred", [C, H, W], dt.float32, kind="ExternalInput")
    target = nc.dram_tensor("target", [H, W], dt.int32, kind="ExternalInput")
    out = nc.dram_tensor("out", [1, 32], dt.float32, kind="ExternalOutput")

    with tile.TileContext(nc) as tc, ExitStack() as ctx:
        consts = ctx.enter_context(tc.tile_pool(name="consts", bufs=1))
        keep = ctx.enter_context(tc.tile_pool(name="keep", bufs=1))
        mp = ctx.enter_context(tc.tile_pool(name="maskpool", bufs=1))
        ppool = ctx.enter_context(tc.tile_pool(name="pp", bufs=4))
        epool = ctx.enter_context(tc.tile_pool(name="ep", bufs=4))
        qpool = ctx.enter_context(tc.tile_pool(name="qp", bufs=2))
        jpool = ctx.enter_context(tc.tile_pool(name="jp", bufs=2))
        opool = ctx.enter_context(tc.tile_pool(name="op", bufs=4))
        fin = ctx.enter_context(tc.tile_pool(name="fin", bufs=1))
        mps = ctx.enter_context(tc.tile_pool(name="mpsum", bufs=1,
                                             space="PSUM"))
        sgp = ctx.enter_context(tc.tile_pool(name="sgpsum", bufs=1,
                                             space="PSUM"))

        # ---------------- pred stream: issue ALL chunk DMAs up front ------
        # class 20 first (small fill chunk), class 19 last (small tail
        # chunk); the target load rides the same sync ring right after the
        # first chunk so nothing delays the stream start
        chunk_lists = [[20], [0]] + [[c, c + 1] for c in range(1, 18, 2)]             + [[19]]
        p_tiles = []
        t32b = mp.tile([128, G4, W], dt.int32, tag="t32b")
        for k, cl in enumerate(chunk_lists):
            nct = len(cl)
            p_t = ppool.tile([128, nct, G4, W], dt.float32, tag="p")
            nc.sync.dma_start(
                out=p_t,
                in_=pred.ap()[cl[0]:cl[0] + nct].rearrange(
                    "c (p r) w -> p c r w", p=128))
            p_tiles.append(p_t)
            if k == 0:
                nc.sync.dma_start(
                    out=t32b,
                    in_=target.ap().rearrange("(p r) w -> p r w", p=128))

        # layout-B tensors
        tb = keep.tile([128, G4, W], dt.float16)      # target as fp16
        maskb = keep.tile([128, G4, W], dt.float16)   # mask
        g_sb = keep.tile([128, 1, W], dt.float16)     # r=3 gather | 1.0
        # cast runs on DVE during its idle head
        nc.vector.tensor_copy(out=tb, in_=t32b)

        # ---------------- constants ----------------
        ident = consts.tile([128, 128], dt.float16)
        make_identity(nc, ident)
        ones = consts.tile([128, 1], dt.float32)
        nc.gpsimd.memset(ones, 1.0)
        st_m = consts.tile([128, 2], dt.float32)
        w1acc = consts.tile([128, 1], dt.float32)
        l2acc = consts.tile([128, 2], dt.float32)
        nc.gpsimd.memset(g_sb, 1.0)

        # banded weights: W[p, i] = 1 iff (i - p) in [lo, hi]
        def band(name, lo, hi):
            w = consts.tile([128, 128], dt.float16, tag=name)
            nc.gpsimd.memset(w, 1.0)
            nc.gpsimd.affine_select(
                out=w, in_=w, compare_op=Alu.is_ge, fill=0.0,
                base=-lo, pattern=[[1, 128]], channel_multiplier=-1)
            nc.gpsimd.affine_select(
                out=w, in_=w, compare_op=Alu.is_ge, fill=0.0,
                base=hi, pattern=[[-1, 128]], channel_multiplier=1)
            return w

        # layout-B row bands.  Image row of (p, r) is 4p+r.  For delta_r =
        # r_in - r_out, the partition band v = p_out - p_in ... solved:
        # horizontal-pair window rows [R-2, R+2]: 4v in [dr-2, dr+2]
        # vertical-pair window rows [R-2, R+1]:   4v in [dr-1, dr+2]
        bands = {}

        def get_band(lo, hi):
            if (lo, hi) not in bands:
                bands[(lo, hi)] = band(f"b{lo}_{hi}", lo, hi)
            return bands[(lo, hi)]

        def vrange(lo4, hi4):
            import math
            lo = math.ceil(lo4 / 4)
            hi = math.floor(hi4 / 4)
            return lo, hi

        B5 = {dr: vrange(dr - 2, dr + 2) for dr in range(-3, 4)}
        B4 = {dr: vrange(dr - 1, dr + 2) for dr in range(-3, 4)}

        # ---------------- mask pipeline tiles (layout B) ----------------
        eh = mp.tile([128, G4, EW], dt.float16, tag="eh")
        ev = mp.tile([128, G4, EW], dt.float16, tag="ev")
        s2 = mp.tile([128, G4, EW], dt.float16, tag="s2")
        s4 = mp.tile([128, G4, EW], dt.float16, tag="s4")
        cs4 = mp.tile([128, G4, EW], dt.float16, tag="cs4")
        cs5 = mp.tile([128, G4, W], dt.float16, tag="cs5")
        for t in (eh, ev):
            nc.gpsimd.memset(t, 0.0)

        bps = mps.tile([128, 512], dt.float32, tag="bps")

        def st_eh():
            # horizontal adjacency edge map (1.0 where neighbors differ)
            nc.vector.tensor_tensor(
                out=eh[:, :, 2:1 + W], in0=tb[:, :, 0:W - 1],
                in1=tb[:, :, 1:W], op=Alu.not_equal)

        def st_ev():
            # vertical pairs: rows r=0..2 are in-partition; the r=3 pair
            # row (4p+4 = next partition's row 0) comes from a PE shift
            # matmul through the rotating PSUM bank
            nc.vector.tensor_tensor(
                out=ev[:, 0:3, 2:2 + W], in0=tb[:, 0:3, :],
                in1=tb[:, 1:4, :], op=Alu.not_equal)
            nc.tensor.matmul(bps, get_band(-1, -1), tb[:, 0, :],
                             start=True, stop=False)
            # partition 127 has no successor row (image bottom): feed its
            # own row back so the neq comes out 0 there
            w127 = consts.tile([128, 128], dt.float16, tag="w127")
            nc.gpsimd.memset(w127, 1.0)
            nc.gpsimd.affine_select(
                out=w127, in_=w127, compare_op=Alu.is_ge, fill=0.0,
                base=0, pattern=[[1, 128]], channel_multiplier=-1)
            nc.gpsimd.affine_select(
                out=w127, in_=w127, compare_op=Alu.is_ge, fill=0.0,
                base=-0, pattern=[[-1, 128]], channel_multiplier=1)
            nc.gpsimd.affine_select(
                out=w127, in_=w127, compare_op=Alu.is_ge, fill=0.0,
                base=-127, pattern=[[0, 128]], channel_multiplier=1)
            nc.tensor.matmul(bps, w127, tb[:, 3, :],
                             start=False, stop=True)
            nc.vector.tensor_tensor(
                out=ev[:, 3:4, 2:2 + W], in0=tb[:, 3:4, :],
                in1=bps, op=Alu.not_equal)

        def st_cs4():
            # cs4[k] = eh[k..k+3] (pairs j in [c-2, c+1] at col c = idx k)
            nc.vector.tensor_tensor(
                out=s2[:, :, 0:EW - 1], in0=eh[:, :, 0:EW - 1],
                in1=eh[:, :, 1:EW], op=Alu.add)
            nc.vector.tensor_tensor(
                out=cs4[:, :, 0:EW - 3], in0=s2[:, :, 0:EW - 3],
                in1=s2[:, :, 2:EW - 1], op=Alu.add)

        def st_cs5():
            # cs5[c] = ev[c..c+4] (cols c-2..c+2)
            nc.vector.tensor_tensor(
                out=s2[:, :, 0:EW - 1], in0=ev[:, :, 0:EW - 1],
                in1=ev[:, :, 1:EW], op=Alu.add)
            nc.vector.tensor_tensor(
                out=s4[:, :, 0:EW - 3], in0=s2[:, :, 0:EW - 3],
                in1=s2[:, :, 2:EW - 1], op=Alu.add)
            nc.vector.tensor_tensor(
                out=cs5, in0=s4[:, :, 0:W], in1=ev[:, :, 4:4 + W],
                op=Alu.add)

        def st_box(r_out):
            # row-direction banded sums into the rotating PSUM bank, then
            # threshold straight off PSUM into maskb (sign(B) in {0,1})
            def f():
                mms = []
                for r_in in range(4):
                    dr = r_in - r_out
                    mms.append((get_band(*B5[dr]), cs4[:, r_in, 0:W]))
                    mms.append((get_band(*B4[dr]), cs5[:, r_in, :]))
                for idx, (wgt, mov) in enumerate(mms):
                    nc.tensor.matmul(bps, wgt, mov, start=(idx == 0),
                                     stop=(idx == len(mms) - 1))
                nc.scalar.sign(out=maskb[:, r_out, :], in_=bps)
            return f

        def st_msum(h):
            # split into halves so each fits the ACT slack of one chunk
            def f():
                junk_m = mp.tile([128, 2, W], dt.float16, tag="junkm")
                nc.scalar.activation(out=junk_m,
                                     in_=maskb[:, 2 * h:2 * h + 2, :],
                                     func=Act.Copy,
                                     accum_out=st_m[:, h:h + 1])
            return f

        sched = {
            1: [st_eh, st_ev],
            2: [st_cs4, st_cs5],
            3: [st_box(0), st_box(1)],
            4: [st_box(2), st_box(3)],
            7: [st_msum(0)],
            8: [st_msum(1)],
        }

        # ---------------- class loop (layout B), stages interleaved -------
        s_ps = sgp.tile([128, G4, W], dt.float32, tag="s")
        g_ps = sgp.tile([128, 3, W], dt.float32, tag="g")

        first_c = chunk_lists[0][0]
        last_c = chunk_lists[-1][-1]
        eq_last = keep.tile([128, G4, W], dt.uint16)
        for k, cl in enumerate(chunk_lists):
            for st in sched.get(k, []):
                st()
            if k == 9:
                # precompute the tail class's eq during a mid-stream gap
                nc.vector.tensor_scalar(
                    out=eq_last, in0=tb, scalar1=float(last_c), scalar2=None,
                    op0=Alu.is_equal)
            nct = len(cl)
            p_t = p_tiles[k]
            e_t = epool.tile([128, nct, G4, W], dt.float16, tag="e")
            nc.scalar.activation(out=e_t, in_=p_t, func=Act.Exp)
            for i in range(nct):
                c = cl[i]
                if c == last_c:
                    eq_t = eq_last
                else:
                    eq_t = qpool.tile([128, G4, W], dt.uint16, tag="q")
                    nc.vector.tensor_scalar(
                        out=eq_t, in0=tb, scalar1=float(c), scalar2=None,
                        op0=Alu.is_equal)
                # rows 0..2: gather via multiply + identity matmul
                o_t = opool.tile([128, 3, W], dt.float16, tag="o")
                nc.vector.tensor_tensor(
                    out=o_t, in0=eq_t[:, 0:3, :], in1=e_t[:, i, 0:3, :],
                    op=Alu.mult)
                # row 3: gather via predicated overwrite (1x but quarter-FD)
                nc.vector.copy_predicated(out=g_sb[:, 0, :],
                                          mask=eq_t[:, 3, :],
                                          data=e_t[:, i, 3, :])
                for j in range(4):
                    nc.tensor.matmul(
                        s_ps[:, j, :], ident, e_t[:, i, j, :],
                        start=(c == first_c), stop=(c == last_c))
                for j in range(3):
                    nc.tensor.matmul(
                        g_ps[:, j, :], ident, o_t[:, j, :],
                        start=(c == first_c), stop=(c == last_c))

        # ---------------- finals ----------------
        l1 = fin.tile([128, G4, W], dt.float16, tag="l1")
        nc.scalar.activation(out=l1, in_=s_ps, func=Act.Ln)
        j1 = jpool.tile([128, G4, W], dt.float16, tag="junkw")
        nc.vector.scalar_tensor_tensor(
            out=j1, in0=l1, scalar=0.0, in1=maskb,
            op0=Alu.add, op1=Alu.mult, accum_out=w1acc[:, 0:1])
        lg = fin.tile([128, 3, W], dt.float16, tag="lg")
        nc.scalar.activation(out=lg, in_=g_ps, func=Act.Ln)
        j2 = jpool.tile([128, 3, W], dt.float16, tag="junkl")
        nc.vector.scalar_tensor_tensor(
            out=j2, in0=lg, scalar=0.0, in1=maskb[:, 0:3, :],
            op0=Alu.add, op1=Alu.mult, accum_out=l2acc[:, 0:1])
        lg4 = fin.tile([128, 1, W], dt.float16, tag="lg4")
        nc.scalar.activation(out=lg4, in_=g_sb, func=Act.Ln)
        j3 = jpool.tile([128, 1, W], dt.float16, tag="junk4")
        nc.vector.scalar_tensor_tensor(
            out=j3, in0=lg4, scalar=0.0, in1=maskb[:, 3:4, :],
            op0=Alu.add, op1=Alu.mult, accum_out=l2acc[:, 1:2])

        # partition reductions — reuse the S bank (fully consumed by l1)
        red = s_ps[0:1, 0, 0:32]
        nc.tensor.matmul(red[:, 0:1], ones, w1acc[:, 0:1], start=True,
                         stop=True)
        nc.tensor.matmul(red[:, 4:6], ones, l2acc[:, 0:2], start=True,
                         stop=True)
        nc.tensor.matmul(red[:, 8:10], ones, st_m, start=True, stop=True)
        outsb = consts.tile([1, 32], dt.float32)
        nc.vector.memset(outsb, 0.0)
        nc.vector.tensor_copy(out=outsb[:, 0:10], in_=red[:, 0:10])
        nc.sync.dma_start(out=out.ap(), in_=outsb)

    nc.compile()
    return nc


def get_nc():
    if "nc" not in _CACHE:
        _CACHE["nc"] = _build_nc()
    return _CACHE["nc"]


def _combine(outs):
    """outs: list of per-core [1,32] float32 -> scalar loss."""
    per_sample = []
    for o in outs:
        w1 = float(o[0, 0])
        l2 = float(o[0, 4:6].sum())
        msum = float(o[0, 8:10].sum())
        wsum = w1 - l2
        if msum > 0:
            per_sample.append(wsum / max(msum, 1.0))
        else:
            per_sample.append(wsum / float(H * W))
    return np.float32(np.mean(per_sample))


def kernel(pred, target):
    from concourse.bass_utils import run_bass_kernel_spmd

    pred = np.ascontiguousarray(pred, dtype=np.float32)
    target = np.ascontiguousarray(target, dtype=np.int32)
    assert pred.shape == (B, C, H, W) and target.shape == (B, H, W)

    nc = get_nc()
    in_maps = [{"pred": pred[b], "target": target[b]} for b in range(B)]
    res = run_bass_kernel_spmd(nc, in_maps, core_ids=list(range(N_CORES)))
    outs = [res.results[b]["out"] for b in range(B)]
    return np.asarray(_combine(outs), dtype=np.float32)


# revision 33
# speedup vs baseline: 1.1882x; 1.1677x over previous
"""Trainium2 Bass kernel for BoundaryLoss (data-parallel over batch).

Math (per batch sample b):
  mask  = boundary mask of target = (maxpool5x5(t) != minpool5x5(t)) with
          cv2-style clipped windows (OOB ignored).  Equals the reference's
          per-class dilate/erode union because a 5x5 window is non-uniform
          iff some class boundary passes through it.
  ce    = logsumexp_c(pred) - pred[t]
  wsum  = sum(mask * ce);  msum = sum(mask)
  per_sample = msum > 0 ? wsum/max(msum,1) : wsum/(H*W);  out = mean_b

Device algorithm (one sample per core), v3:
  - pred streams in "layout B" [128, (4 rows, 512)] (partition p = rows
    4p..4p+3) giving 8KB-contiguous DMA runs.  The pred chunk DMAs are
    the only traffic on the sync HWDGE ring and are issued first; both
    target loads are casting SWDGE DMAs on gpsimd (int32->fp16), so no
    compute engine spends time on casts and the pred stream starts at
    t~=0.
  - S = sum_c exp(pred_c): exp on ACT (fp16 out), summed over classes by
    identity-matmul PSUM accumulation on TensorE (4 banks).
  - picked = pred[t]: per class, eq=(t==c) on DVE 4x; rows 0-2 gathered
    via eq*e mult + identity-matmul PSUM accumulation (3 banks); row 3
    via copy_predicated into SBUF (quarter-FD but 1 op).
  - boundary mask via adjacency edge maps (NO transposes, NO min/max
    pools): a 5x5 window is uniform iff no horizontal-adjacent pair
    differs in rows[r+-2] x cols[c-2..c+1] and no vertical-adjacent pair
    differs in rows[r-2..r+1] x cols[c+-2].  eh/ev neq maps + column box
    sums on DVE (shifted adds); row box sums as banded-weight matmuls
    into one rotating PSUM bank; mask = Sign(B) on ACT straight from
    PSUM.  Row-shift for ev and the layout A->B bounce ride SWDGE.
  - finals per row-group: Ln(S), Ln(G) on ACT; mask-weighted stt accums
    on DVE; partition-reduce via ones-matmuls; DMA out [1,32].
Host combines the per-core outputs.
"""

import numpy as np

B = 8
C = 21
H = 512
W = 512
N_CORES = 8
CHUNK = 2  # pred planes per DMA
EW = 516  # padded width of edge-map tiles; data cols [2, 514)
G4 = 4  # row groups (H = G4 * 128)

_CACHE = {}


def _build_nc():
    from contextlib import ExitStack

    import concourse.bacc as bacc
    import concourse.tile as tile
    from concourse import mybir
    from concourse.masks import make_identity

    dt = mybir.dt
    Alu = mybir.AluOpType
    Act = mybir.ActivationFunctionType

    nc = bacc.Bacc("TRN2", target_bir_lowering=False, debug=False,
                   num_devices=N_CORES)

    pred = nc.dram_tensor("pred", [C, H, W], dt.float32, kind="ExternalInput")
    target = nc.dram_tensor("target", [H, W], dt.int32, kind="ExternalInput")
    out = nc.dram_tensor("out", [1, 32], dt.float32, kind="ExternalOutput")

    with tile.TileContext(nc) as tc, ExitStack() as ctx:
        consts = ctx.enter_context(tc.tile_pool(name="consts", bufs=1))
        keep = ctx.enter_context(tc.tile_pool(name="keep", bufs=1))
        mp = ctx.enter_context(tc.tile_pool(name="maskpool", bufs=1))
        ppool = ctx.enter_context(tc.tile_pool(name="pp", bufs=4))
        epool = ctx.enter_context(tc.tile_pool(name="ep", bufs=4))
        qpool = ctx.enter_context(tc.tile_pool(name="qp", bufs=2))
        jpool = ctx.enter_context(tc.tile_pool(name="jp", bufs=2))
        opool = ctx.enter_context(tc.tile_pool(name="op", bufs=4))
        fin = ctx.enter_context(tc.tile_pool(name="fin", bufs=1))
        mps = ctx.enter_context(tc.tile_pool(name="mpsum", bufs=1,
                                             space="PSUM"))
        sgp = ctx.enter_context(tc.tile_pool(name="sgpsum", bufs=1,
                                             space="PSUM"))

        # ---------------- pred stream: issue ALL chunk DMAs up front ------
        # class 20 first (small fill chunk), class 19 last (small tail
        # chunk); the target load rides the same sync ring right after the
        # first chunk so nothing delays the stream start
        chunk_lists = [[20], [0]] + [[c, c + 1] for c in range(1, 18, 2)]             + [[19]]
        p_tiles = []
        t32b = mp.tile([128, G4, W], dt.int32, tag="t32b")
        for k, cl in enumerate(chunk_lists):
            nct = len(cl)
            p_t = ppool.tile([128, nct, G4, W], dt.float32, tag="p")
            nc.sync.dma_start(
                out=p_t,
                in_=pred.ap()[cl[0]:cl[0] + nct].rearrange(
                    "c (p r) w -> p c r w", p=128))
            p_tiles.append(p_t)
            if k == 0:
                nc.sync.dma_start(
                    out=t32b,
                    in_=target.ap().rearrange("(p r) w -> p r w", p=128))

        # layout-B tensors
        tb = keep.tile([128, G4, W], dt.float16)      # target as fp16
        maskb = keep.tile([128, G4, W], dt.float16)   # mask
        g_sb = keep.tile([128, 1, W], dt.float16)     # r=3 gather | 1.0
        # cast runs on DVE during its idle head
        nc.vector.tensor_copy(out=tb, in_=t32b)

        # ---------------- constants ----------------
        ident = consts.tile([128, 128], dt.float16)
        make_identity(nc, ident)
        ones = consts.tile([128, 1], dt.float32)
        nc.gpsimd.memset(ones, 1.0)
        st_m = consts.tile([128, 2], dt.float32)
        w1acc = consts.tile([128, 1], dt.float32)
        l2acc = consts.tile([128, 2], dt.float32)
        nc.gpsimd.memset(g_sb, 1.0)

        # banded weights: W[p, i] = 1 iff (i - p) in [lo, hi]
        def band(name, lo, hi):
            w = consts.tile([128, 128], dt.float16, tag=name)
            nc.gpsimd.memset(w, 1.0)
            nc.gpsimd.affine_select(
                out=w, in_=w, compare_op=Alu.is_ge, fill=0.0,
                base=-lo, pattern=[[1, 128]], channel_multiplier=-1)
            nc.gpsimd.affine_select(
                out=w, in_=w, compare_op=Alu.is_ge, fill=0.0,
                base=hi, pattern=[[-1, 128]], channel_multiplier=1)
            return w

        # layout-B row bands.  Image row of (p, r) is 4p+r.  For delta_r =
        # r_in - r_out, the partition band v = p_out - p_in ... solved:
        # horizontal-pair window rows [R-2, R+2]: 4v in [dr-2, dr+2]
        # vertical-pair window rows [R-2, R+1]:   4v in [dr-1, dr+2]
        bands = {}

        def get_band(lo, hi):
            if (lo, hi) not in bands:
                bands[(lo, hi)] = band(f"b{lo}_{hi}", lo, hi)
            return bands[(lo, hi)]

        def vrange(lo4, hi4):
            import math
            lo = math.ceil(lo4 / 4)
            hi = math.floor(hi4 / 4)
            return lo, hi

        B5 = {dr: vrange(dr - 2, dr + 2) for dr in range(-3, 4)}
        B4 = {dr: vrange(dr - 1, dr + 2) for dr in range(-3, 4)}

        # ---------------- mask pipeline tiles (layout B) ----------------
        eh = mp.tile([128, G4, EW], dt.float16, tag="eh")
        ev = mp.tile([128, G4, EW], dt.float16, tag="ev")
        s2 = mp.tile([128, G4, EW], dt.float16, tag="s2")
        s4 = mp.tile([128, G4, EW], dt.float16, tag="s4")
        cs4 = mp.tile([128, G4, EW], dt.float16, tag="cs4")
        cs5 = mp.tile([128, G4, W], dt.float16, tag="cs5")
        for t in (eh, ev):
            nc.gpsimd.memset(t, 0.0)

        bps = mps.tile([128, 512], dt.float32, tag="bps")

        def st_eh():
            # horizontal adjacency edge map (1.0 where neighbors differ)
            nc.vector.tensor_tensor(
                out=eh[:, :, 2:1 + W], in0=tb[:, :, 0:W - 1],
                in1=tb[:, :, 1:W], op=Alu.not_equal)

        def st_ev():
            # vertical pairs: rows r=0..2 are in-partition; the r=3 pair
            # row (4p+4 = next partition's row 0) comes from a PE shift
            # matmul through the rotating PSUM bank
            nc.vector.tensor_tensor(
                out=ev[:, 0:3, 2:2 + W], in0=tb[:, 0:3, :],
                in1=tb[:, 1:4, :], op=Alu.not_equal)
            nc.tensor.matmul(bps, get_band(-1, -1), tb[:, 0, :],
                             start=True, stop=False)
            # partition 127 has no successor row (image bottom): feed its
            # own row back so the neq comes out 0 there
            w127 = consts.tile([128, 128], dt.float16, tag="w127")
            nc.gpsimd.memset(w127, 1.0)
            nc.gpsimd.affine_select(
                out=w127, in_=w127, compare_op=Alu.is_ge, fill=0.0,
                base=0, pattern=[[1, 128]], channel_multiplier=-1)
            nc.gpsimd.affine_select(
                out=w127, in_=w127, compare_op=Alu.is_ge, fill=0.0,
                base=-0, pattern=[[-1, 128]], channel_multiplier=1)
            nc.gpsimd.affine_select(
                out=w127, in_=w127, compare_op=Alu.is_ge, fill=0.0,
                base=-127, pattern=[[0, 128]], channel_multiplier=1)
            nc.tensor.matmul(bps, w127, tb[:, 3, :],
                             start=False, stop=True)
            nc.vector.tensor_tensor(
                out=ev[:, 3:4, 2:2 + W], in0=tb[:, 3:4, :],
                in1=bps, op=Alu.not_equal)

        def st_cs4():
            # cs4[k] = eh[k..k+3] (pairs j in [c-2, c+1] at col c = idx k)
            nc.vector.tensor_tensor(
                out=s2[:, :, 0:EW - 1], in0=eh[:, :, 0:EW - 1],
                in1=eh[:, :, 1:EW], op=Alu.add)
            nc.vector.tensor_tensor(
                out=cs4[:, :, 0:EW - 3], in0=s2[:, :, 0:EW - 3],
                in1=s2[:, :, 2:EW - 1], op=Alu.add)

        def st_cs5():
            # cs5[c] = ev[c..c+4] (cols c-2..c+2)
            nc.vector.tensor_tensor(
                out=s2[:, :, 0:EW - 1], in0=ev[:, :, 0:EW - 1],
                in1=ev[:, :, 1:EW], op=Alu.add)
            nc.vector.tensor_tensor(
                out=s4[:, :, 0:EW - 3], in0=s2[:, :, 0:EW - 3],
                in1=s2[:, :, 2:EW - 1], op=Alu.add)
            nc.vector.tensor_tensor(
                out=cs5, in0=s4[:, :, 0:W], in1=ev[:, :, 4:4 + W],
                op=Alu.add)

        def st_box(r_out):
            # row-direction banded sums into the rotating PSUM bank, then
            # threshold straight off PSUM into maskb (sign(B) in {0,1})
            def f():
                mms = []
                for r_in in range(4):
                    dr = r_in - r_out
                    mms.append((get_band(*B5[dr]), cs4[:, r_in, 0:W]))
                    mms.append((get_band(*B4[dr]), cs5[:, r_in, :]))
                for idx, (wgt, mov) in enumerate(mms):
                    nc.tensor.matmul(bps, wgt, mov, start=(idx == 0),
                                     stop=(idx == len(mms) - 1))
                nc.scalar.sign(out=maskb[:, r_out, :], in_=bps)
            return f

        def st_msum(h):
            # split into halves so each fits the ACT slack of one chunk
            def f():
                junk_m = mp.tile([128, 2, W], dt.float16, tag="junkm")
                nc.scalar.activation(out=junk_m,
                                     in_=maskb[:, 2 * h:2 * h + 2, :],
                                     func=Act.Copy,
                                     accum_out=st_m[:, h:h + 1])
            return f

        sched = {
            1: [st_eh, st_ev],
            2: [st_cs4, st_cs5],
            3: [st_box(0), st_box(1)],
            4: [st_box(2), st_box(3)],
            7: [st_msum(0)],
            8: [st_msum(1)],
        }

        # ---------------- class loop (layout B), stages interleaved -------
        s_ps = sgp.tile([128, G4, W], dt.float32, tag="s")
        g_ps = sgp.tile([128, 3, W], dt.float32, tag="g")

        first_c = chunk_lists[0][0]
        last_c = chunk_lists[-1][-1]
        for k, cl in enumerate(chunk_lists):
            for st in sched.get(k, []):
                st()
            nct = len(cl)
            p_t = p_tiles[k]
            e_t = epool.tile([128, nct, G4, W], dt.float16, tag="e")
            nc.scalar.activation(out=e_t, in_=p_t, func=Act.Exp)
            for i in range(nct):
                c = cl[i]
                eq_t = qpool.tile([128, G4, W], dt.uint16, tag="q")
                nc.vector.tensor_scalar(
                    out=eq_t, in0=tb, scalar1=float(c), scalar2=None,
                    op0=Alu.is_equal)
                # rows 0..2: gather via multiply + identity matmul
                o_t = opool.tile([128, 3, W], dt.float16, tag="o")
                nc.vector.tensor_tensor(
                    out=o_t, in0=eq_t[:, 0:3, :], in1=e_t[:, i, 0:3, :],
                    op=Alu.mult)
                # row 3: gather via predicated overwrite (1x but quarter-FD)
                nc.vector.copy_predicated(out=g_sb[:, 0, :],
                                          mask=eq_t[:, 3, :],
                                          data=e_t[:, i, 3, :])
                for j in range(4):
                    nc.tensor.matmul(
                        s_ps[:, j, :], ident, e_t[:, i, j, :],
                        start=(c == first_c), stop=(c == last_c))
                for j in range(3):
                    nc.tensor.matmul(
                        g_ps[:, j, :], ident, o_t[:, j, :],
                        start=(c == first_c), stop=(c == last_c))

        # ---------------- finals ----------------
        l1 = fin.tile([128, G4, W], dt.float16, tag="l1")
        nc.scalar.activation(out=l1, in_=s_ps, func=Act.Ln)
        j1 = jpool.tile([128, G4, W], dt.float16, tag="junkw")
        nc.vector.scalar_tensor_tensor(
            out=j1, in0=l1, scalar=0.0, in1=maskb,
            op0=Alu.add, op1=Alu.mult, accum_out=w1acc[:, 0:1])
        lg = fin.tile([128, 3, W], dt.float16, tag="lg")
        nc.scalar.activation(out=lg, in_=g_ps, func=Act.Ln)
        j2 = jpool.tile([128, 3, W], dt.float16, tag="junkl")
        nc.vector.scalar_tensor_tensor(
            out=j2, in0=lg, scalar=0.0, in1=maskb[:, 0:3, :],
            op0=Alu.add, op1=Alu.mult, accum_out=l2acc[:, 0:1])
        lg4 = fin.tile([128, 1, W], dt.float16, tag="lg4")
        nc.scalar.activation(out=lg4, in_=g_sb, func=Act.Ln)
        j3 = jpool.tile([128, 1, W], dt.float16, tag="junk4")
        nc.vector.scalar_tensor_tensor(
            out=j3, in0=lg4, scalar=0.0, in1=maskb[:, 3:4, :],
            op0=Alu.add, op1=Alu.mult, accum_out=l2acc[:, 1:2])

        # partition reductions — reuse the S bank (fully consumed by l1)
        red = s_ps[0:1, 0, 0:32]
        nc.tensor.matmul(red[:, 0:1], ones, w1acc[:, 0:1], start=True,
                         stop=True)
        nc.tensor.matmul(red[:, 4:6], ones, l2acc[:, 0:2], start=True,
                         stop=True)
        nc.tensor.matmul(red[:, 8:10], ones, st_m, start=True, stop=True)
        outsb = consts.tile([1, 32], dt.float32)
        nc.vector.memset(outsb, 0.0)
        nc.vector.tensor_copy(out=outsb[:, 0:10], in_=red[:, 0:10])
        nc.sync.dma_start(out=out.ap(), in_=outsb)

    nc.compile()
    return nc


def get_nc():
    if "nc" not in _CACHE:
        _CACHE["nc"] = _build_nc()
    return _CACHE["nc"]


def _combine(outs):
    """outs: list of per-core [1,32] float32 -> scalar loss."""
    per_sample = []
    for o in outs:
        w1 = float(o[0, 0])
        l2 = float(o[0, 4:6].sum())
        msum = float(o[0, 8:10].sum())
        wsum = w1 - l2
        if msum > 0:
            per_sample.append(wsum / max(msum, 1.0))
        else:
            per_sample.append(wsum / float(H * W))
    return np.float32(np.mean(per_sample))


def kernel(pred, target):
    from concourse.bass_utils import run_bass_kernel_spmd

    pred = np.ascontiguousarray(pred, dtype=np.float32)
    target = np.ascontiguousarray(target, dtype=np.int32)
    assert pred.shape == (B, C, H, W) and target.shape == (B, H, W)

    nc = get_nc()
    in_maps = [{"pred": pred[b], "target": target[b]} for b in range(B)]
    res = run_bass_kernel_spmd(nc, in_maps, core_ids=list(range(N_CORES)))
    outs = [res.results[b]["out"] for b in range(B)]
    return np.asarray(_combine(outs), dtype=np.float32)
